# revision 4
# baseline (speedup 1.0000x reference)
"""Causal multi-head self-attention on 8 Trainium2 NeuronCores.

Problem: B=8, T=1024, D=1024, 16 heads (H=64), fp32, causal softmax,
y = softmax(mask(q k^T)/sqrt(H)) v, then output projection. Weights are
nn.Linear style: q = x @ Wq^T etc.

Sharding: pure data-parallel — one batch element per core, weights
replicated, no collectives.

Per-core layout strategy (all feature-major, zero on-device transposes):
  host sends xT = x[b].T  [d, t]  and W*T = W*.T  [d_in, d_out]
  qT[do,t] = sum_d WqT[d,do] * xT[d,t]   (lhsT=WqT, rhs=xT)
  kT       likewise
  v[t,do]  = sum_d xT[d,t]  * WvT[d,do]  (lhsT=xT,  rhs=WvT) -> natural layout
  per head h:  S^T[tk,tq] = sum_hd kT_h[hd,tk] qT_h[hd,tq]
               E = exp(S^T/8) * causal_mask   (mask multiplicative, no -inf)
               outX[m,tq] = sum_tk Vplus_h[tk,m] E[tk,tq],  Vplus = [v_h | 1]
               outT_h = outX[0:64] * (1/outX[64])           (flash-style denom)
  y[t,do] = sum_d outT[d,t] * WoT[d,do]  (lhsT=outT, rhs=WoT)
"""

import numpy as np
from contextlib import ExitStack

N_CORES = 8
T = 1024
D = 1024
NH = 16
HD = 64  # head dim
P = 128
NT = D // P  # 8 tiles of 128 along d or t
NQ = 2       # tq tiles of 512
TQ = 512
SCALE = 1.0 / 8.0  # 1/sqrt(64)

# matmul compute dtype: "fp32" (exact, 1/4 rate) or "fp32r" (TF32-like, full rate)
MM_MODE = "fp32"

_COMPILED = {}


def _build(nc, tile, mybir, mm_dt):
    """Emit the per-core Tile program into nc."""
    f32 = mybir.dt.float32
    Exp = mybir.ActivationFunctionType.Exp

    xT_d = nc.declare_dram_parameter("xT", [D, T], f32, isOutput=False)
    wqT_d = nc.declare_dram_parameter("wqT", [D, D], f32, isOutput=False)
    wkT_d = nc.declare_dram_parameter("wkT", [D, D], f32, isOutput=False)
    wvT_d = nc.declare_dram_parameter("wvT", [D, D], f32, isOutput=False)
    woT_d = nc.declare_dram_parameter("woT", [D, D], f32, isOutput=False)
    msk_d = nc.declare_dram_parameter("mask", [P, 4 * TQ], f32, isOutput=False)
    y_d = nc.declare_dram_parameter("y", [T, D], f32, isOutput=True)

    def mm(out, lhsT, rhs, start, stop):
        nc.tensor.matmul(
            out, lhsT.bitcast(mm_dt), rhs.bitcast(mm_dt), start=start, stop=stop
        )

    with ExitStack() as ctx:
        tc = ctx.enter_context(tile.TileContext(nc))

        # ---- resident pools (live across all phases) ----
        pqk = ctx.enter_context(tc.tile_pool(name="pqk", bufs=16))
        pv = ctx.enter_context(tc.tile_pool(name="pv", bufs=8))
        pmask = ctx.enter_context(tc.tile_pool(name="pmask", bufs=1))
        # psum pools (8 banks total: 4 + 2 used)
        pp_a = ctx.enter_context(tc.tile_pool(name="pp_a", bufs=4, space="PSUM"))
        pp_o = ctx.enter_context(tc.tile_pool(name="pp_o", bufs=2, space="PSUM"))

        mask_sb = pmask.tile([P, 4 * TQ], f32)
        nc.sync.dma_start(out=mask_sb[:], in_=msk_d[:])

        qT = [pqk.tile([P, T], f32, tag="qk", name=f"qT{i}") for i in range(NT)]
        kT = [pqk.tile([P, T], f32, tag="qk", name=f"kT{i}") for i in range(NT)]
        # v-plus layout: head h at cols [65h .. 65h+63], ones col at 65h+64
        VP = HD + 1
        vp = [pv.tile([P, NH * VP], f32, tag="vp", name=f"vp{i}") for i in range(NT)]

        # ---- phase 1: projections ----
        with tc.tile_pool(name="pw", bufs=10) as pw:
            with tc.tile_pool(name="pxt", bufs=8) as pxt:
                xT = [pxt.tile([P, T], f32, tag="xt", name=f"xTs{i}") for i in range(NT)]
                for k in range(NT):
                    nc.sync.dma_start(
                        out=xT[k][:], in_=xT_d[k * P : (k + 1) * P, :]
                    )
                # ones in v-plus (set before v eviction overwrites head cols)
                for m in range(NT):
                    nc.vector.memset(vp[m][:], 1.0)

                for w_d, dst in [(wqT_d, qT), (wkT_d, kT), (wvT_d, None)]:
                    w_sb = [pw.tile([P, D], f32, tag="w", name=f"w{i}") for i in range(NT)]
                    for k in range(NT):
                        nc.sync.dma_start(
                            out=w_sb[k][:], in_=w_d[k * P : (k + 1) * P, :]
                        )
                    for m in range(NT):
                        for n in range(NQ):
                            ps = pp_a.tile([P, TQ], f32)
                            for k in range(NT):
                                if dst is not None:
                                    # qT/kT: lhsT=W^T tile, rhs=xT tile
                                    mm(
                                        ps[:],
                                        w_sb[k][:, m * P : (m + 1) * P],
                                        xT[k][:, n * TQ : (n + 1) * TQ],
                                        start=(k == 0),
                                        stop=(k == NT - 1),
                                    )
                                else:
                                    # v: lhsT=xT tile, rhs=W^T tile -> natural
                                    mm(
                                        ps[:],
                                        xT[k][:, m * P : (m + 1) * P],
                                        w_sb[k][:, n * TQ : (n + 1) * TQ],
                                        start=(k == 0),
                                        stop=(k == NT - 1),
                                    )
                            if dst is not None:
                                nc.vector.tensor_copy(
                                    dst[m][:, n * TQ : (n + 1) * TQ], ps[:]
                                )
                            else:
                                # scatter 8 head slices into v-plus layout
                                for hh in range(8):
                                    h = n * 8 + hh
                                    nc.vector.tensor_copy(
                                        vp[m][:, h * VP : h * VP + HD],
                                        ps[:, hh * HD : (hh + 1) * HD],
                                    )

        # outT opens after phase-1 pools close (stack alloc reuses space)
        pout = ctx.enter_context(tc.tile_pool(name="pout", bufs=8))
        outT = [pout.tile([P, T], f32, tag="ot", name=f"outT{i}") for i in range(NT)]

        # ---- phase 2: attention, head pairs interleaved ----
        with (
            tc.tile_pool(name="pe", bufs=16) as pe,
            tc.tile_pool(name="pnrm", bufs=4) as pnrm,
            tc.tile_pool(name="pbc", bufs=4) as pbc,
        ):
            for pr in range(NH // 2):
                it = pr  # qT/kT/outT tile index for this pair
                for j in range(NQ):
                    ni = 4 * j + 4  # tk tiles needed: i < ni
                    # scores + exp for both heads of the pair, interleaved
                    etiles = {}
                    for i in range(ni):
                        for half in range(2):
                            po = half * HD
                            ps = pp_a.tile([P, TQ], f32)
                            mm(
                                ps[:],
                                kT[it][po : po + HD, i * P : (i + 1) * P],
                                qT[it][po : po + HD, j * TQ : (j + 1) * TQ],
                                start=True,
                                stop=True,
                            )
                            e = pe.tile([P, TQ], f32, tag="e")
                            nc.scalar.activation(e[:], ps[:], Exp, scale=SCALE)
                            g = i - 4 * j
                            if g >= 0:  # diagonal block: multiplicative mask
                                nc.vector.tensor_mul(
                                    e[:], e[:], mask_sb[:, g * TQ : (g + 1) * TQ]
                                )
                            etiles[(half, i)] = e
                    # att @ V-plus (M=65: 64 head dims + denominator row)
                    for half in range(2):
                        h = 2 * pr + half
                        po_ps = pp_o.tile([HD + 1, TQ], f32)
                        for i in range(ni):
                            mm(
                                po_ps[:],
                                vp[i][:, h * VP : h * VP + VP],
                                etiles[(half, i)][:],
                                start=(i == 0),
                                stop=(i == ni - 1),
                            )
                        # normalize rows 0..63 by reciprocal of denom row 64,
                        # broadcast across partitions via stride-0 DMA
                        rt = pnrm.tile([1, TQ], f32, tag="rt")
                        nc.vector.reciprocal(rt[:], po_ps[HD : HD + 1, :])
                        bt = pbc.tile([HD, TQ], f32, tag="bt")
                        nc.gpsimd.partition_broadcast(bt[:], rt[:])
                        if half == 0:
                            # aligned: write straight into outT partitions 0..63
                            nc.vector.tensor_mul(
                                outT[it][0:HD, j * TQ : (j + 1) * TQ],
                                po_ps[0:HD, :],
                                bt[:],
                            )
                        else:
                            # outT partitions 64..127: cross-partition write,
                            # so mul into tmp then DMA partition-shift
                            nt_ = pnrm.tile([HD, TQ], f32, tag="nt")
                            nc.vector.tensor_mul(
                                nt_[:], po_ps[0:HD, :], bt[:]
                            )
                            nc.sync.dma_start(
                                out=outT[it][HD:P, j * TQ : (j + 1) * TQ],
                                in_=nt_[:],
                            )

        # ---- phase 3: output projection ----
        with (
            tc.tile_pool(name="pw2", bufs=8) as pw2,
            tc.tile_pool(name="py", bufs=4) as py,
        ):
            wo_sb = [pw2.tile([P, D], f32, tag="wo", name=f"wo{i}") for i in range(NT)]
            for k in range(NT):
                nc.sync.dma_start(out=wo_sb[k][:], in_=woT_d[k * P : (k + 1) * P, :])
            for m in range(NT):
                for n in range(NQ):
                    ps = pp_a.tile([P, TQ], f32)
                    for k in range(NT):
                        mm(
                            ps[:],
                            outT[k][:, m * P : (m + 1) * P],
                            wo_sb[k][:, n * TQ : (n + 1) * TQ],
                            start=(k == 0),
                            stop=(k == NT - 1),
                        )
                    ysb = py.tile([P, TQ], f32, tag="y")
                    nc.vector.tensor_copy(ysb[:], ps[:])
                    nc.sync.dma_start(
                        out=y_d[m * P : (m + 1) * P, n * TQ : (n + 1) * TQ],
                        in_=ysb[:],
                    )
    return nc


def build_program(mm_mode=None):
    """Build + compile the SPMD program once; returns the Bacc object."""
    mode = mm_mode or MM_MODE
    if mode in _COMPILED:
        return _COMPILED[mode]
    import concourse.bacc as bacc
    import concourse.tile as tile
    from concourse import mybir

    mm_dt = mybir.dt.float32 if mode == "fp32" else mybir.dt.float32r
    nc = bacc.Bacc("TRN2", target_bir_lowering=False, debug=False,
                   num_devices=N_CORES)
    _build(nc, tile, mybir, mm_dt)
    nc.compile()
    _COMPILED[mode] = nc
    return nc


def make_mask():
    # mask[p, g*512 + f] = 1 if f >= p + 128*g  (block offset g = i - 4j)
    p = np.arange(P)[:, None]
    f = np.arange(TQ)[None, :]
    cols = [(f >= p + P * g).astype(np.float32) for g in range(4)]
    return np.ascontiguousarray(np.concatenate(cols, axis=1))


def make_in_maps(x, Wk, Wq, Wv, Wo):
    wqT = np.ascontiguousarray(np.asarray(Wq, dtype=np.float32).T)
    wkT = np.ascontiguousarray(np.asarray(Wk, dtype=np.float32).T)
    wvT = np.ascontiguousarray(np.asarray(Wv, dtype=np.float32).T)
    woT = np.ascontiguousarray(np.asarray(Wo, dtype=np.float32).T)
    msk = make_mask()
    in_maps = []
    for b in range(N_CORES):
        in_maps.append({
            "xT": np.ascontiguousarray(x[b].T),
            "wqT": wqT, "wkT": wkT, "wvT": wvT, "woT": woT,
            "mask": msk,
        })
    return in_maps


def kernel(x, Wk, Wq, Wv, Wo):
    from concourse.bass_utils import run_bass_kernel_spmd

    x = np.asarray(x, dtype=np.float32)
    nc = build_program()
    in_maps = make_in_maps(x, Wk, Wq, Wv, Wo)
    res = run_bass_kernel_spmd(nc, in_maps, list(range(N_CORES)))
    return np.stack([res.results[c]["y"] for c in range(N_CORES)], axis=0)


# revision 6
# speedup vs baseline: 1.5973x; 1.5973x over previous
"""Causal multi-head self-attention on 8 Trainium2 NeuronCores.

Problem: B=8, T=1024, D=1024, 16 heads (H=64), fp32, causal softmax,
y = softmax(mask(q k^T)/sqrt(H)) v, then output projection. Weights are
nn.Linear style: q = x @ Wq^T etc.

Sharding: pure data-parallel — one batch element per core, weights
replicated, no collectives.

Per-core layout strategy (all feature-major, zero on-device transposes):
  host sends xT = x[b].T  [d, t]  and W*T = W*.T  [d_in, d_out]
  qT[do,t] = sum_d WqT[d,do] * xT[d,t]   (lhsT=WqT, rhs=xT)
  kT       likewise
  v[t,do]  = sum_d xT[d,t]  * WvT[d,do]  (lhsT=xT,  rhs=WvT) -> natural layout
  per head h:  S^T[tk,tq] = sum_hd kT_h[hd,tk] qT_h[hd,tq]
               E = exp(S^T/8) * causal_mask   (mask multiplicative, no -inf)
               outX[m,tq] = sum_tk Vplus_h[tk,m] E[tk,tq],  Vplus = [v_h | 1]
               outT_h = outX[0:64] * (1/outX[64])           (flash-style denom)
  y[t,do] = sum_d outT[d,t] * WoT[d,do]  (lhsT=outT, rhs=WoT)
"""

import numpy as np
from contextlib import ExitStack

N_CORES = 8
T = 1024
D = 1024
NH = 16
HD = 64  # head dim
P = 128
NT = D // P  # 8 tiles of 128 along d or t
NQ = 2       # tq tiles of 512
TQ = 512
SCALE = 1.0 / 8.0  # 1/sqrt(64)

# matmul compute dtype: "fp32" (exact, 1/4 rate) or "fp32r" (TF32-like, full rate)
MM_MODE = "fp32"

_COMPILED = {}


def _build(nc, tile, mybir, mm_dt):
    """Emit the per-core Tile program into nc."""
    f32 = mybir.dt.float32
    Exp = mybir.ActivationFunctionType.Exp

    mdt = mm_dt  # storage dtype for matmul operands (f32 or f32r)
    xT_d = nc.declare_dram_parameter("xT", [D, T], mdt, isOutput=False)
    wqT_d = nc.declare_dram_parameter("wqT", [D, D], mdt, isOutput=False)
    wkT_d = nc.declare_dram_parameter("wkT", [D, D], mdt, isOutput=False)
    wvT_d = nc.declare_dram_parameter("wvT", [D, D], mdt, isOutput=False)
    woT_d = nc.declare_dram_parameter("woT", [D, D], mdt, isOutput=False)
    msk_d = nc.declare_dram_parameter("mask", [P, 4 * TQ], mdt, isOutput=False)
    ones_d = nc.declare_dram_parameter("ones", [P, NH], mdt, isOutput=False)
    y_d = nc.declare_dram_parameter("y", [T, D], f32, isOutput=True)

    def mm(out, lhsT, rhs, start, stop):
        nc.tensor.matmul(out, lhsT, rhs, start=start, stop=stop)

    with ExitStack() as ctx:
        tc = ctx.enter_context(tile.TileContext(nc))

        # ---- resident pools (live across all phases) ----
        pqk = ctx.enter_context(tc.tile_pool(name="pqk", bufs=16))
        pv = ctx.enter_context(tc.tile_pool(name="pv", bufs=8))
        pmask = ctx.enter_context(tc.tile_pool(name="pmask", bufs=1))
        # psum pools (8 banks total: 4 + 2 used)
        pp_a = ctx.enter_context(tc.tile_pool(name="pp_a", bufs=4, space="PSUM"))
        pp_o = ctx.enter_context(tc.tile_pool(name="pp_o", bufs=2, space="PSUM"))

        mask_sb = pmask.tile([P, 4 * TQ], mdt)
        nc.sync.dma_start(out=mask_sb[:], in_=msk_d[:])

        qT = [pqk.tile([P, T], mdt, tag="qk", name=f"qT{i}") for i in range(NT)]
        kT = [pqk.tile([P, T], mdt, tag="qk", name=f"kT{i}") for i in range(NT)]
        # v-plus layout: head h at cols [65h .. 65h+63], ones col at 65h+64
        VP = HD + 1
        vp = [pv.tile([P, NH * VP], mdt, tag="vp", name=f"vp{i}") for i in range(NT)]

        # ---- phase 1: projections ----
        with tc.tile_pool(name="pw", bufs=10) as pw:
            with tc.tile_pool(name="pxt", bufs=8) as pxt:
                xT = [pxt.tile([P, T], mdt, tag="xt", name=f"xTs{i}") for i in range(NT)]
                for k in range(NT):
                    nc.sync.dma_start(
                        out=xT[k][:], in_=xT_d[k * P : (k + 1) * P, :]
                    )
                # ones columns of v-plus via DMA (memset can't write f32r)
                for m in range(NT):
                    ones_cols = vp[m].rearrange("p (h c) -> p h c", c=VP)[:, :, VP - 1]
                    nc.sync.dma_start(out=ones_cols, in_=ones_d[:])

                for w_d, dst in [(wqT_d, qT), (wkT_d, kT), (wvT_d, None)]:
                    w_sb = [pw.tile([P, D], mdt, tag="w", name=f"w{i}") for i in range(NT)]
                    for k in range(NT):
                        nc.sync.dma_start(
                            out=w_sb[k][:], in_=w_d[k * P : (k + 1) * P, :]
                        )
                    for m in range(NT):
                        for n in range(NQ):
                            ps = pp_a.tile([P, TQ], f32)
                            for k in range(NT):
                                if dst is not None:
                                    # qT/kT: lhsT=W^T tile, rhs=xT tile
                                    mm(
                                        ps[:],
                                        w_sb[k][:, m * P : (m + 1) * P],
                                        xT[k][:, n * TQ : (n + 1) * TQ],
                                        start=(k == 0),
                                        stop=(k == NT - 1),
                                    )
                                else:
                                    # v: lhsT=xT tile, rhs=W^T tile -> natural
                                    mm(
                                        ps[:],
                                        xT[k][:, m * P : (m + 1) * P],
                                        w_sb[k][:, n * TQ : (n + 1) * TQ],
                                        start=(k == 0),
                                        stop=(k == NT - 1),
                                    )
                            if dst is not None:
                                nc.vector.tensor_copy(
                                    dst[m][:, n * TQ : (n + 1) * TQ], ps[:]
                                )
                            else:
                                # scatter 8 head slices into v-plus layout
                                for hh in range(8):
                                    h = n * 8 + hh
                                    nc.vector.tensor_copy(
                                        vp[m][:, h * VP : h * VP + HD],
                                        ps[:, hh * HD : (hh + 1) * HD],
                                    )

        # outT opens after phase-1 pools close (stack alloc reuses space)
        pout = ctx.enter_context(tc.tile_pool(name="pout", bufs=8))
        outT = [pout.tile([P, T], mdt, tag="ot", name=f"outT{i}") for i in range(NT)]

        # ---- phase 2: attention, head pairs interleaved ----
        with (
            tc.tile_pool(name="pe", bufs=16) as pe,
            tc.tile_pool(name="pnrm", bufs=4) as pnrm,
            tc.tile_pool(name="pbc", bufs=4) as pbc,
        ):
            for pr in range(NH // 2):
                it = pr  # qT/kT/outT tile index for this pair
                for j in range(NQ):
                    ni = 4 * j + 4  # tk tiles needed: i < ni
                    # scores + exp for both heads of the pair, interleaved
                    etiles = {}
                    for i in range(ni):
                        for half in range(2):
                            po = half * HD
                            ps = pp_a.tile([P, TQ], f32)
                            mm(
                                ps[:],
                                kT[it][po : po + HD, i * P : (i + 1) * P],
                                qT[it][po : po + HD, j * TQ : (j + 1) * TQ],
                                start=True,
                                stop=True,
                            )
                            e = pe.tile([P, TQ], mdt, tag="e")
                            nc.scalar.activation(e[:], ps[:], Exp, scale=SCALE)
                            g = i - 4 * j
                            if g >= 0:  # diagonal block: multiplicative mask
                                nc.vector.tensor_mul(
                                    e[:], e[:], mask_sb[:, g * TQ : (g + 1) * TQ]
                                )
                            etiles[(half, i)] = e
                    # att @ V-plus (M=65: 64 head dims + denominator row)
                    for half in range(2):
                        h = 2 * pr + half
                        po_ps = pp_o.tile([HD + 1, TQ], f32)
                        for i in range(ni):
                            mm(
                                po_ps[:],
                                vp[i][:, h * VP : h * VP + VP],
                                etiles[(half, i)][:],
                                start=(i == 0),
                                stop=(i == ni - 1),
                            )
                        # normalize rows 0..63 by reciprocal of denom row 64,
                        # broadcast across partitions via stride-0 DMA
                        rt = pnrm.tile([1, TQ], f32, tag="rt")
                        nc.vector.reciprocal(rt[:], po_ps[HD : HD + 1, :])
                        bt = pbc.tile([HD, TQ], f32, tag="bt")
                        nc.gpsimd.partition_broadcast(bt[:], rt[:])
                        if half == 0:
                            # aligned: write straight into outT partitions 0..63
                            nc.vector.tensor_mul(
                                outT[it][0:HD, j * TQ : (j + 1) * TQ],
                                po_ps[0:HD, :],
                                bt[:],
                            )
                        else:
                            # outT partitions 64..127: cross-partition write,
                            # so mul into tmp then DMA partition-shift
                            nt_ = pnrm.tile([HD, TQ], mdt, tag="nt")
                            nc.vector.tensor_mul(
                                nt_[:], po_ps[0:HD, :], bt[:]
                            )
                            nc.sync.dma_start(
                                out=outT[it][HD:P, j * TQ : (j + 1) * TQ],
                                in_=nt_[:],
                            )

        # ---- phase 3: output projection ----
        with (
            tc.tile_pool(name="pw2", bufs=8) as pw2,
            tc.tile_pool(name="py", bufs=4) as py,
        ):
            wo_sb = [pw2.tile([P, D], mdt, tag="wo", name=f"wo{i}") for i in range(NT)]
            for k in range(NT):
                nc.sync.dma_start(out=wo_sb[k][:], in_=woT_d[k * P : (k + 1) * P, :])
            for m in range(NT):
                for n in range(NQ):
                    ps = pp_a.tile([P, TQ], f32)
                    for k in range(NT):
                        mm(
                            ps[:],
                            outT[k][:, m * P : (m + 1) * P],
                            wo_sb[k][:, n * TQ : (n + 1) * TQ],
                            start=(k == 0),
                            stop=(k == NT - 1),
                        )
                    ysb = py.tile([P, TQ], f32, tag="y")
                    nc.vector.tensor_copy(ysb[:], ps[:])
                    nc.sync.dma_start(
                        out=y_d[m * P : (m + 1) * P, n * TQ : (n + 1) * TQ],
                        in_=ysb[:],
                    )
    return nc


def build_program(mm_mode=None):
    """Build + compile the SPMD program once; returns the Bacc object."""
    mode = mm_mode or MM_MODE
    if mode in _COMPILED:
        return _COMPILED[mode]
    import concourse.bacc as bacc
    import concourse.tile as tile
    from concourse import mybir

    mm_dt = mybir.dt.float32 if mode == "fp32" else mybir.dt.float32r
    nc = bacc.Bacc("TRN2", target_bir_lowering=False, debug=False,
                   num_devices=N_CORES)
    _build(nc, tile, mybir, mm_dt)
    nc.compile()
    _COMPILED[mode] = nc
    return nc


def make_mask():
    # mask[p, g*512 + f] = 1 if f >= p + 128*g  (block offset g = i - 4j)
    p = np.arange(P)[:, None]
    f = np.arange(TQ)[None, :]
    cols = [(f >= p + P * g).astype(np.float32) for g in range(4)]
    return np.ascontiguousarray(np.concatenate(cols, axis=1))


def make_in_maps(x, Wk, Wq, Wv, Wo):
    wqT = np.ascontiguousarray(np.asarray(Wq, dtype=np.float32).T)
    wkT = np.ascontiguousarray(np.asarray(Wk, dtype=np.float32).T)
    wvT = np.ascontiguousarray(np.asarray(Wv, dtype=np.float32).T)
    woT = np.ascontiguousarray(np.asarray(Wo, dtype=np.float32).T)
    msk = make_mask()
    in_maps = []
    for b in range(N_CORES):
        in_maps.append({
            "xT": np.ascontiguousarray(x[b].T),
            "wqT": wqT, "wkT": wkT, "wvT": wvT, "woT": woT,
            "mask": msk, "ones": np.ones((P, NH), np.float32),
        })
    return in_maps


def kernel(x, Wk, Wq, Wv, Wo):
    from concourse.bass_utils import run_bass_kernel_spmd

    x = np.asarray(x, dtype=np.float32)
    nc = build_program()
    in_maps = make_in_maps(x, Wk, Wq, Wv, Wo)
    res = run_bass_kernel_spmd(nc, in_maps, list(range(N_CORES)))
    return np.stack([res.results[c]["y"] for c in range(N_CORES)], axis=0)


# revision 10
# speedup vs baseline: 1.9084x; 1.1947x over previous
"""Causal multi-head self-attention on 8 Trainium2 NeuronCores.

Problem: B=8, T=1024, D=1024, 16 heads (H=64), fp32, causal softmax,
y = softmax(mask(q k^T)/sqrt(H)) v, then output projection. Weights are
nn.Linear style: q = x @ Wq^T etc.

Sharding: pure data-parallel — one batch element per core, weights
replicated, no collectives.

Per-core layout strategy (all feature-major, zero on-device transposes):
  host sends xT = x[b].T  [d, t]  and W*T = W*.T  [d_in, d_out]
  qT[do,t] = sum_d WqT[d,do] * xT[d,t]   (lhsT=WqT, rhs=xT)
  kT       likewise
  v[t,do]  = sum_d xT[d,t]  * WvT[d,do]  (lhsT=xT,  rhs=WvT) -> natural layout
  per head pair (2p, 2p+1), per tq block of 512, per tk block of 128:
    S^T[tk,tq]   = sum_hd kT_h[hd,tk] qT_h[hd,tq]   (both heads into one
                   [128,1024] PSUM superblock, one 512-col half per head)
    diag blocks:  S^T += (-3200*I).T @ imask_g      (PE additive causal mask)
    E = exp(S^T/8)                                  (one ACT op per superblock)
    outX[m,tq]   = sum_tk Vplus_h[tk,m] E_h[tk,tq], Vplus_h = [v_h | 1]
    outT_h       = outX[0:64] * recip(outX[64])     (flash-style denominator;
                   reciprocals batched 8 rows at a time, broadcast across
                   partitions via a DRAM-bounce DMA)
  y[t,do] = sum_d outT[d,t] * WoT[d,do]  (lhsT=outT, rhs=WoT)
"""

import numpy as np
from contextlib import ExitStack

N_CORES = 8
T = 1024
D = 1024
NH = 16
HD = 64  # head dim
P = 128
NT = D // P  # 8 tiles of 128 along d or t
NQ = 2       # tq tiles of 512
TQ = 512
SCALE = 1.0 / 8.0  # 1/sqrt(64)
MASKVAL = -3200.0  # additive causal mask; *SCALE = -400 -> exp -> 0

# matmul compute dtype: "fp32" (exact, 1/4 rate) or "fp32r" (TF32-like, full rate)
MM_MODE = "fp32"

_COMPILED = {}


def _build(nc, tile, mybir, mm_dt):
    """Emit the per-core Tile program into nc."""
    f32 = mybir.dt.float32
    Exp = mybir.ActivationFunctionType.Exp

    mdt = mm_dt  # storage dtype for matmul operands (f32 or f32r)
    xT_d = nc.declare_dram_parameter("xT", [D, T], mdt, isOutput=False)
    wqT_d = nc.declare_dram_parameter("wqT", [D, D], mdt, isOutput=False)
    wkT_d = nc.declare_dram_parameter("wkT", [D, D], mdt, isOutput=False)
    wvT_d = nc.declare_dram_parameter("wvT", [D, D], mdt, isOutput=False)
    woT_d = nc.declare_dram_parameter("woT", [D, D], mdt, isOutput=False)
    msk_d = nc.declare_dram_parameter("imask", [P, 4 * TQ], mdt, isOutput=False)
    negi_d = nc.declare_dram_parameter("negi", [P, P], mdt, isOutput=False)
    ones_d = nc.declare_dram_parameter("ones", [P, NH], mdt, isOutput=False)
    y_d = nc.declare_dram_parameter("y", [T, D], f32, isOutput=True)

    nrm_d = nc.dram_tensor("nrm_scratch", [8, 8, TQ], f32)

    def mm(out, lhsT, rhs, start, stop):
        nc.tensor.matmul(out, lhsT, rhs, start=start, stop=stop)

    with ExitStack() as ctx:
        tc = ctx.enter_context(tile.TileContext(nc))

        # ---- resident pools ----
        pqk = ctx.enter_context(tc.tile_pool(name="pqk", bufs=16))
        pv = ctx.enter_context(tc.tile_pool(name="pv", bufs=8))
        pmask = ctx.enter_context(tc.tile_pool(name="pmask", bufs=1))
        # psum: 3 superblock slots (2 banks each) + 2 attV slots = 8 banks
        pp_big = ctx.enter_context(
            tc.tile_pool(name="pp_big", bufs=3, space="PSUM")
        )
        pp_o = ctx.enter_context(tc.tile_pool(name="pp_o", bufs=2, space="PSUM"))

        mask_sb = pmask.tile([P, 4 * TQ], mdt)
        nc.sync.dma_start(out=mask_sb[:], in_=msk_d[:])
        negi_sb = pmask.tile([P, P], mdt)
        nc.sync.dma_start(out=negi_sb[:], in_=negi_d[:])

        qT = [pqk.tile([P, T], mdt, tag="qk", name=f"qT{i}") for i in range(NT)]
        kT = [pqk.tile([P, T], mdt, tag="qk", name=f"kT{i}") for i in range(NT)]
        # v-plus layout: head h at cols [65h .. 65h+63], ones col at 65h+64
        VP = HD + 1
        vp = [pv.tile([P, NH * VP], mdt, tag="vp", name=f"vp{i}") for i in range(NT)]

        # ---- phase 1: projections ----
        with tc.tile_pool(name="pw", bufs=10) as pw:
            with tc.tile_pool(name="pxt", bufs=8) as pxt:
                xT = [pxt.tile([P, T], mdt, tag="xt", name=f"xTs{i}")
                      for i in range(NT)]
                for k in range(NT):
                    nc.sync.dma_start(
                        out=xT[k][:], in_=xT_d[k * P : (k + 1) * P, :]
                    )
                # ones columns of v-plus via DMA (memset can't write f32r)
                for m in range(NT):
                    ones_cols = vp[m].rearrange(
                        "p (h c) -> p h c", c=VP
                    )[:, :, VP - 1]
                    nc.sync.dma_start(out=ones_cols, in_=ones_d[:])

                for w_d, dst in [(wqT_d, qT), (wkT_d, kT), (wvT_d, None)]:
                    w_sb = [pw.tile([P, D], mdt, tag="w", name=f"w{i}")
                            for i in range(NT)]
                    for k in range(NT):
                        nc.sync.dma_start(
                            out=w_sb[k][:], in_=w_d[k * P : (k + 1) * P, :]
                        )
                    for m in range(NT):
                        for n in range(NQ):
                            ps = pp_big.tile([P, TQ], f32, tag="ps")
                            for k in range(NT):
                                if dst is not None:
                                    mm(ps[:],
                                       w_sb[k][:, m * P : (m + 1) * P],
                                       xT[k][:, n * TQ : (n + 1) * TQ],
                                       start=(k == 0), stop=(k == NT - 1))
                                else:
                                    mm(ps[:],
                                       xT[k][:, m * P : (m + 1) * P],
                                       w_sb[k][:, n * TQ : (n + 1) * TQ],
                                       start=(k == 0), stop=(k == NT - 1))
                            if dst is not None:
                                nc.vector.tensor_copy(
                                    dst[m][:, n * TQ : (n + 1) * TQ], ps[:]
                                )
                            else:
                                # one strided copy scatters 8 head slices
                                vdst = vp[m].rearrange(
                                    "p (h c) -> p h c", c=VP
                                )[:, n * 8 : (n + 1) * 8, 0:HD]
                                vsrc = ps.rearrange("p (h c) -> p h c", c=HD)
                                nc.vector.tensor_copy(vdst, vsrc)

        # outT opens after phase-1 pools close (stack alloc reuses space)
        pout = ctx.enter_context(tc.tile_pool(name="pout", bufs=8))
        outT = [pout.tile([P, T], mdt, tag="ot", name=f"outT{i}")
                for i in range(NT)]

        # ---- phase 2: attention ----
        with (
            tc.tile_pool(name="pe", bufs=9) as pe,
            tc.tile_pool(name="pux", bufs=4) as pux,
            tc.tile_pool(name="pct", bufs=1) as pct,
            tc.tile_pool(name="pbc", bufs=2) as pbc,
        ):
            pending = []  # (ux, it, j, half) awaiting normalize
            state = {"flush_no": 0}

            def flush():
                if not pending:
                    return
                fno = state["flush_no"]
                nb = len(pending)
                ct = pct.tile([4, TQ], f32, tag="ct")
                for r, (ux, _, _, _) in enumerate(pending):
                    nc.sync.dma_start(
                        out=ct[r : r + 1, :], in_=ux[HD : HD + 1, :]
                    )
                nc.vector.reciprocal(ct[:nb, :], ct[:nb, :])
                nc.sync.dma_start(out=nrm_d[fno, :nb, :], in_=ct[:nb, :])
                for r, (ux, it, j, half) in enumerate(pending):
                    bt = pbc.tile([HD, TQ], f32, tag="bt")
                    nc.sync.dma_start(
                        out=bt[:],
                        in_=nrm_d[fno, r : r + 1, :].to_broadcast([HD, TQ]),
                    )
                    if half == 0:
                        nc.vector.tensor_mul(
                            outT[it][0:HD, j * TQ : (j + 1) * TQ],
                            ux[0:HD, :], bt[:],
                        )
                    else:
                        nt_ = pbc.tile([HD, TQ], mdt, tag="nt")
                        nc.vector.tensor_mul(nt_[:], ux[0:HD, :], bt[:])
                        nc.sync.dma_start(
                            out=outT[it][HD:P, j * TQ : (j + 1) * TQ],
                            in_=nt_[:],
                        )
                pending.clear()
                state["flush_no"] = fno + 1

            for pr in range(NH // 2):
                it = pr  # qT/kT/outT tile index for this pair
                for j in range(NQ):
                    ni = 4 * j + 4  # tk tiles needed: i < ni
                    esup = []
                    for i in range(ni):
                        ps = pp_big.tile([P, 2 * TQ], f32, tag="ps")
                        g = i - 4 * j
                        for half in range(2):
                            po = half * HD
                            c = half * TQ
                            mm(ps[:, c : c + TQ],
                               kT[it][po : po + HD, i * P : (i + 1) * P],
                               qT[it][po : po + HD, j * TQ : (j + 1) * TQ],
                               start=True, stop=(g < 0))
                            if g >= 0:  # additive causal mask on PE
                                mm(ps[:, c : c + TQ],
                                   negi_sb[:],
                                   mask_sb[:, g * TQ : (g + 1) * TQ],
                                   start=False, stop=True)
                        e = pe.tile([P, 2 * TQ], mdt, tag="e")
                        nc.scalar.activation(e[:], ps[:], Exp, scale=SCALE)
                        esup.append(e)
                    for half in range(2):
                        h = 2 * pr + half
                        po_ps = pp_o.tile([HD + 1, TQ], f32)
                        for i in range(ni):
                            mm(po_ps[:],
                               vp[i][:, h * VP : h * VP + VP],
                               esup[i][:, half * TQ : (half + 1) * TQ],
                               start=(i == 0), stop=(i == ni - 1))
                        ux = pux.tile([HD + 1, TQ], f32, tag="ux")
                        nc.vector.tensor_copy(ux[:], po_ps[:])
                        pending.append((ux, it, j, half))
                flush()

        # ---- phase 3: output projection ----
        with (
            tc.tile_pool(name="pw2", bufs=8) as pw2,
            tc.tile_pool(name="py", bufs=4) as py,
        ):
            wo_sb = [pw2.tile([P, D], mdt, tag="wo", name=f"wo{i}")
                     for i in range(NT)]
            for k in range(NT):
                nc.sync.dma_start(out=wo_sb[k][:], in_=woT_d[k * P : (k + 1) * P, :])
            for m in range(NT):
                for n in range(NQ):
                    ps = pp_big.tile([P, TQ], f32, tag="ps")
                    for k in range(NT):
                        mm(ps[:],
                           outT[k][:, m * P : (m + 1) * P],
                           wo_sb[k][:, n * TQ : (n + 1) * TQ],
                           start=(k == 0), stop=(k == NT - 1))
                    ysb = py.tile([P, TQ], f32, tag="y")
                    nc.vector.tensor_copy(ysb[:], ps[:])
                    nc.sync.dma_start(
                        out=y_d[m * P : (m + 1) * P, n * TQ : (n + 1) * TQ],
                        in_=ysb[:],
                    )
    return nc


def build_program(mm_mode=None):
    """Build + compile the SPMD program once; returns the Bacc object."""
    mode = mm_mode or MM_MODE
    if mode in _COMPILED:
        return _COMPILED[mode]
    import concourse.bacc as bacc
    import concourse.tile as tile
    from concourse import mybir

    mm_dt = mybir.dt.float32 if mode == "fp32" else mybir.dt.float32r
    nc = bacc.Bacc("TRN2", target_bir_lowering=False, debug=False,
                   num_devices=N_CORES)
    _build(nc, tile, mybir, mm_dt)
    nc.compile()
    _COMPILED[mode] = nc
    return nc


def make_mask():
    # inverse mask: imask[p, g*512 + f] = 1 where MASKED (tk > tq), i.e.
    # f < p + 128*g   (block column offset g = i - 4j)
    p = np.arange(P)[:, None]
    f = np.arange(TQ)[None, :]
    cols = [(f < p + P * g).astype(np.float32) for g in range(4)]
    return np.ascontiguousarray(np.concatenate(cols, axis=1))


def make_in_maps(x, Wk, Wq, Wv, Wo):
    wqT = np.ascontiguousarray(np.asarray(Wq, dtype=np.float32).T)
    wkT = np.ascontiguousarray(np.asarray(Wk, dtype=np.float32).T)
    wvT = np.ascontiguousarray(np.asarray(Wv, dtype=np.float32).T)
    woT = np.ascontiguousarray(np.asarray(Wo, dtype=np.float32).T)
    msk = make_mask()
    negi = np.ascontiguousarray(MASKVAL * np.eye(P, dtype=np.float32))
    ones = np.ones((P, NH), np.float32)
    in_maps = []
    for b in range(N_CORES):
        in_maps.append({
            "xT": np.ascontiguousarray(x[b].T),
            "wqT": wqT, "wkT": wkT, "wvT": wvT, "woT": woT,
            "imask": msk, "negi": negi, "ones": ones,
        })
    return in_maps


def kernel(x, Wk, Wq, Wv, Wo):
    from concourse.bass_utils import run_bass_kernel_spmd

    x = np.asarray(x, dtype=np.float32)
    nc = build_program()
    in_maps = make_in_maps(x, Wk, Wq, Wv, Wo)
    res = run_bass_kernel_spmd(nc, in_maps, list(range(N_CORES)))
    return np.stack([res.results[c]["y"] for c in range(N_CORES)], axis=0)
